# revision 1
# baseline (speedup 1.0000x reference)
"""Causal self-attention (B=4, T=2048, D=1024, H=16) on 8 TRN2 NeuronCores.

Sharding: core i = (batch b = i//2, head-group g = i%2). Data parallel on B,
tensor parallel on heads (8 heads per group): qkv_proj columns and out_proj
rows split per head group. Each core computes a partial [D, T] output^T for
its batch; host sums the two group partials per batch, transposes, adds bias.

Per-core pipeline (all matmuls in float32r = FP22, full PE rate at N>=256):
  phase 1: x -> x^T via PE transpose; V = x@Wv (natural [t,d] + ones col);
           Q^T, K^T = (x@Wq)^T via transposed projection, bounced to DRAM.
  phase 2: per head pair p, per q-chunk qc (512), per k-tile j (128):
           S^T[k,q] = K^T.T @ Q^T (heads at partitions 0-63 / 64-127);
           one exp over both heads' strips (trimmed to the causal columns);
           triangle mask-mul on the diagonal 128-block; AV: psum[65,512] +=
           V'[k,d+1].T @ P^T accumulated over j -- row 64 is the softmax
           denominator (ones column). Normalize with reciprocal_approx_fast
           + gpsimd partition_broadcast.
  phase 3: out^T[f,t] = sum_p Wo_pair[d128,f].T @ O^T_pair[d128,t].
"""

import numpy as np

import concourse.bacc as bacc
import concourse.tile as tile
import concourse.mybir as mybir
from concourse import bass_utils
from concourse.bass import ts

F32 = mybir.dt.float32
F32R = mybir.dt.float32r
EXP = mybir.ActivationFunctionType.Exp

T = 2048
TT = 16          # t tiles of 128
NP = 4           # head pairs per core
NQC = 4          # q chunks of 512
SCALE = 0.125    # 1/sqrt(64)

_CACHE = {}
_last_in_maps = None


def _build(CT):
    """CT = number of 128-row c-tiles in the (possibly bias-augmented) x/W."""
    nc = bacc.Bacc("TRN2", target_bir_lowering=False, debug=False)
    C = CT * 128

    # keep PE instructions in emission order: the scheduler otherwise
    # interleaves S/AV/proj matmuls, paying a ~250ns array-reconfig penalty
    # on every K=64 <-> K=128 transition
    from concourse.bass import _add_dep_helper

    _pe_last = [None]

    def _chain(inst):
        _pe_last[0] = inst
        return inst

    def mm(*args, **kwargs):
        return _chain(nc.tensor.matmul(*args, **kwargs))

    def mmt(*args, **kwargs):
        return _chain(nc.tensor.transpose(*args, **kwargs))

    xa = nc.dram_tensor("xa", [T, C], F32, kind="ExternalInput").ap()
    wq = nc.dram_tensor("wq", [C, 512], F32, kind="ExternalInput").ap()
    wk = nc.dram_tensor("wk", [C, 512], F32, kind="ExternalInput").ap()
    wv = nc.dram_tensor("wv", [C, 512], F32, kind="ExternalInput").ap()
    wo = nc.dram_tensor("wo", [512, 1024], F32, kind="ExternalInput").ap()
    tri = nc.dram_tensor("tri", [128, 128], F32, kind="ExternalInput").ap()
    idn = nc.dram_tensor("idn", [128, 128], F32, kind="ExternalInput").ap()
    ot = nc.dram_tensor("ot", [1024, T], F32, kind="ExternalOutput").ap()

    with tile.TileContext(nc) as tc:
        with (
            tc.tile_pool(name="persist", bufs=1) as persist,
            tc.tile_pool(name="dram", bufs=1, space="DRAM") as dpool,
        ):
            vS = persist.tile([128, TT, 8, 65], F32R)     # [k128, ktile, head, d+1]
            OT = persist.tile([128, NP, T], F32R)         # [d128(2 heads), pair, t]
            tr = persist.tile([128, 128], F32R)
            wo_sb = persist.tile([128, NP, 1024], F32R)
            nc.vector.memset(vS[:, :, :, 64:65].bitcast(F32), 1.0)

            # Q^T / K^T bounce chunks, one DRAM tile per (pair, t-chunk) so a
            # chunk becomes readable as soon as its projection lands
            qtd = {}
            ktd = {}
            for _p in range(NP):
                for _tc in range(4):
                    qtd[(_p, _tc)] = dpool.tile([128, 512], F32, name=f"qtd{_p}{_tc}")
                    ktd[(_p, _tc)] = dpool.tile([128, 512], F32, name=f"ktd{_p}{_tc}")

            # ---------------- phase 1: transpose + projections ----------------
            with (
                tc.tile_pool(name="ph1", bufs=1) as ph1,
                tc.tile_pool(name="xnat", bufs=6) as xnat,
                tc.tile_pool(name="bounce", bufs=4) as bpool,
                tc.tile_pool(name="pst", bufs=2, space="PSUM") as pst,
                tc.tile_pool(name="psp", bufs=6, space="PSUM") as psp,
            ):
                ident = ph1.tile([128, 128], F32)
                nc.sync.dma_start(out=ident, in_=idn)
                wv_sb = ph1.tile([128, CT, 512], F32R)
                wq_sb = ph1.tile([128, CT, NP, 128], F32R)
                wk_sb = ph1.tile([128, CT, NP, 128], F32R)
                xT = [ph1.tile([128, T], F32R, name=f"xT{cc}") for cc in range(CT)]

                def load_weights():
                    # big strided loads on the gpsimd queue set, emitted after
                    # the first transpose batch so x tiles go out first
                    nc.gpsimd.dma_start(out=tr, in_=tri.bitcast(F32R))
                    nc.gpsimd.dma_start(
                        out=wv_sb,
                        in_=wv.rearrange("(ct P) f -> P ct f", P=128).bitcast(F32R),
                    )
                    nc.gpsimd.dma_start(
                        out=wq_sb,
                        in_=wq.rearrange("(ct P) (np f) -> P ct np f", P=128, np=NP).bitcast(F32R),
                    )
                    nc.gpsimd.dma_start(
                        out=wk_sb,
                        in_=wk.rearrange("(ct P) (np f) -> P ct np f", P=128, np=NP).bitcast(F32R),
                    )
                    nc.gpsimd.dma_start(
                        out=wo_sb,
                        in_=wo.rearrange("(np P) f -> P np f", P=128).bitcast(F32R),
                    )

                def transpose_tt(tt):
                    for cc in range(CT):
                        xn = xnat.tile([128, 128], F32)
                        nc.sync.dma_start(out=xn, in_=xa[ts(tt, 128), ts(cc, 128)])
                        pt_ = pst.tile([128, 512], F32)
                        mmt(pt_[:, :128], xn, ident)
                        nc.vector.tensor_copy(out=xT[cc][:, ts(tt, 128)], in_=pt_[:, :128])

                def vproj_tt(tt):
                    ps = psp.tile([128, 512], F32)
                    for cc in range(CT):
                        mm(
                            ps,
                            lhsT=xT[cc][:, ts(tt, 128)],
                            rhs=wv_sb[:, cc, :],
                            start=(cc == 0),
                            stop=(cc == CT - 1),
                        )
                    nc.vector.tensor_copy(
                        out=vS[:, tt, :, 0:64],
                        in_=ps.rearrange("p (h d) -> p h d", h=8),
                    )

                def qkproj_tc(tc_):
                    for p in range(NP):
                        for w_sb, dst, scl in ((wq_sb, qtd, SCALE), (wk_sb, ktd, 1.0)):
                            ps = psp.tile([128, 512], F32)
                            for cc in range(CT):
                                mm(
                                    ps,
                                    lhsT=w_sb[:, cc, p, :],
                                    rhs=xT[cc][:, ts(tc_, 512)],
                                    start=(cc == 0),
                                    stop=(cc == CT - 1),
                                )
                            bo = bpool.tile([128, 512], F32)
                            nc.scalar.mul(out=bo, in_=ps, mul=scl)
                            nc.sync.dma_start(out=dst[(p, tc_)], in_=bo)

                for tt in range(TT + 1):
                    if tt < TT:
                        transpose_tt(tt)
                    if tt == 0:
                        load_weights()
                    if tt >= 1:
                        vproj_tt(tt - 1)
                        if (tt - 1) % 4 == 3:
                            qkproj_tc((tt - 1) // 4)

            # ---------------- phase 2: attention ----------------
            # Emission keeps the PE in same-type runs: a group of 3 j-steps of
            # S matmuls (+exp on ACT), then the previous group's AV matmuls.
            # Interleaving S/AV per-j costs ~25%/MM in PE streaming rate.
            with (
                tc.tile_pool(name="qkc", bufs=12) as qkcpool,
                tc.tile_pool(name="ptp", bufs=12) as ptpool,
                tc.tile_pool(name="rsm", bufs=4) as rpool,
                tc.tile_pool(name="rbcp", bufs=4) as rbcpool,
                tc.tile_pool(name="psS", bufs=3, space="PSUM") as psS,
                tc.tile_pool(name="psAv", bufs=2, space="PSUM") as psAv,
            ):
                qch = {}
                kch = {}
                avs = {}
                pts = {}

                def fetch_pair(p):
                    for tc_ in range(4):
                        qt = qkcpool.tile([128, 512], F32R, name="qTc", tag="qTc")
                        nc.sync.dma_start(out=qt, in_=qtd[(p, tc_)].bitcast(F32R))
                        qch[(p, tc_)] = qt
                        kt = qkcpool.tile([128, 512], F32R, name="kTc", tag="kTc")
                        nc.sync.dma_start(out=kt, in_=ktd[(p, tc_)].bitcast(F32R))
                        kch[(p, tc_)] = kt

                def s_exp(p, qc, j):
                    off = max(0, 128 * j - 512 * qc)
                    sg = psS.tile([128, 2, 512], F32)
                    kc = kch[(p, j // 4)]
                    qc_t = qch[(p, qc)]
                    jo = 128 * (j % 4)
                    for m in range(2):
                        mm(
                            sg[:, m, off:],
                            lhsT=kc[64 * m : 64 * m + 64, jo : jo + 128],
                            rhs=qc_t[64 * m : 64 * m + 64, off:],
                            start=True,
                            stop=True,
                        )
                    ptile = ptpool.tile([128, 2, 512], F32R)
                    nc.scalar.activation(
                        out=ptile[:, :, off:], in_=sg[:, :, off:], func=EXP
                    )
                    if j >= 4 * qc:
                        nc.vector.tensor_mul(
                            ptile[:, :, off : off + 128],
                            ptile[:, :, off : off + 128],
                            tr[:, None, :].to_broadcast([128, 2, 128]),
                        )
                    pts[(p, qc, j)] = (ptile, off)

                def av_mm(p, qc, j, nj):
                    ptile, off = pts.pop((p, qc, j))
                    av = avs[(p, qc)]
                    for m in range(2):
                        mm(
                            av[m][:65, off:],
                            lhsT=vS[:, j, 2 * p + m, :],
                            rhs=ptile[:, m, off:],
                            start=(j == 0),
                            stop=(j == nj - 1),
                        )

                def normalize(p, qc):
                    av = avs.pop((p, qc))
                    rsbs = []
                    for m in range(2):
                        rsb = rpool.tile([1, 512], F32, name="rsb", tag="rsb")
                        nc.vector.tensor_copy(out=rsb, in_=av[m][64:65, :])
                        # unnormalized O~ out of PSUM so the av bank frees fast
                        nc.vector.tensor_copy(
                            out=OT[64 * m : 64 * m + 64, p, ts(qc, 512)],
                            in_=av[m][0:64, :],
                        )
                        rsbs.append(rsb)
                    for m in range(2):
                        rinv = rpool.tile([1, 512], F32, name="rinv", tag="rinv")
                        nc.vector.reciprocal_approx_fast(out=rinv, in_=rsbs[m])
                        rb = rbcpool.tile([128, 512], F32, name="rb", tag="rb")
                        nc.gpsimd.partition_broadcast(rb, rinv)
                        sl = OT[64 * m : 64 * m + 64, p, ts(qc, 512)]
                        nc.vector.tensor_mul(sl, sl, rb[64 * m : 64 * m + 64, :])

                groups = []
                for p in range(NP):
                    for qc in range(NQC):
                        nj = 4 * qc + 4
                        js = list(range(nj))
                        sub = [js[i : i + 3] for i in range(0, nj, 3)]
                        for gi, jg in enumerate(sub):
                            groups.append((p, qc, nj, jg, gi == 0, gi == len(sub) - 1))

                def av_group(gi):
                    p, qc, nj, jg, first, last = groups[gi]
                    if first:
                        avs[(p, qc)] = [
                            psAv.tile([128, 512], F32, name="av", tag="av")
                            for _ in range(2)
                        ]
                    for j in jg:
                        av_mm(p, qc, j, nj)
                    if last:
                        normalize(p, qc)

                # S-runs of 6 MMs; AV-runs of ~12 (two groups) to amortize the
                # PE row-config switch between K=64 S and K=128 AV matmuls
                LAG = 2
                for i in range(len(groups) + LAG):
                    if i < len(groups):
                        p, qc, nj, jg, first, last = groups[i]
                        if qc == 0 and first:
                            fetch_pair(p)
                        for j in jg:
                            s_exp(p, qc, j)
                    if i >= LAG and (i - LAG) % 2 == 1:
                        av_group(i - LAG - 1)
                        av_group(i - LAG)
                if len(groups) % 2 == 1:
                    av_group(len(groups) - 1)

            # ---------------- phase 3: output projection ----------------
            with (
                tc.tile_pool(name="obnc", bufs=4) as opool,
                tc.tile_pool(name="psO", bufs=8, space="PSUM") as psO,
            ):
                for ft in range(8):
                    pso = [psO.tile([128, 512], F32, name="pso", tag="pso") for _ in range(4)]
                    for p in range(NP):
                        for tc_ in range(4):
                            mm(
                                pso[tc_],
                                lhsT=wo_sb[:, p, ts(ft, 128)],
                                rhs=OT[:, p, ts(tc_, 512)],
                                start=(p == 0),
                                stop=(p == NP - 1),
                            )
                    for tc_ in range(4):
                        ob = opool.tile([128, 512], F32)
                        nc.vector.tensor_copy(out=ob, in_=pso[tc_])
                        nc.sync.dma_start(out=ot[ts(ft, 128), ts(tc_, 512)], in_=ob)

    nc.compile()
    return nc


def kernel(x, W_qkv, b_qkv, W_out, b_out):
    global _last_in_maps
    x = np.asarray(x, dtype=np.float32)
    W_qkv = np.asarray(W_qkv, dtype=np.float32)
    b_qkv = np.asarray(b_qkv, dtype=np.float32)
    W_out = np.asarray(W_out, dtype=np.float32)
    b_out = np.asarray(b_out, dtype=np.float32)
    B = x.shape[0]

    aug = bool(np.any(b_qkv))
    CT = 9 if aug else 8
    if CT not in _CACHE:
        _CACHE[CT] = _build(CT)
    nc = _CACHE[CT]

    # triangle keep-mask for the diagonal 128 block: [p, c] = 1 if c >= p
    tri = (np.arange(128)[None, :] >= np.arange(128)[:, None]).astype(np.float32)

    in_maps = []
    for core in range(8):
        b, g = core // 2, core % 2
        xa = x[b]
        if aug:
            pad = np.zeros((T, 128), np.float32)
            pad[:, 0] = 1.0
            xa = np.concatenate([xa, pad], axis=1)

        def wslice(col0):
            w = W_qkv[:, col0 + 512 * g : col0 + 512 * g + 512]
            if aug:
                extra = np.zeros((128, 512), np.float32)
                extra[0] = b_qkv[col0 + 512 * g : col0 + 512 * g + 512]
                w = np.concatenate([w, extra], axis=0)
            return np.ascontiguousarray(w)

        in_maps.append(
            {
                "xa": np.ascontiguousarray(xa),
                "wq": wslice(0),
                "wk": wslice(1024),
                "wv": wslice(2048),
                "wo": np.ascontiguousarray(W_out[512 * g : 512 * g + 512, :]),
                "tri": tri,
                "idn": np.eye(128, dtype=np.float32),
            }
        )

    _last_in_maps = in_maps
    res = bass_utils.run_bass_kernel_spmd(nc, in_maps, list(range(8))).results
    out = np.empty((B, T, 1024), np.float32)
    for b in range(B):
        acc = res[2 * b]["ot"] + res[2 * b + 1]["ot"]
        out[b] = acc.T + b_out[None, :]
    return out



# revision 5
# speedup vs baseline: 1.4215x; 1.4215x over previous
"""Causal self-attention (B=4, T=2048, D=1024, H=16) on 8 TRN2 NeuronCores.

Sharding: core i = (batch b = i//2, head-group g = i%2). Data parallel on B,
tensor parallel on heads (8 heads per group): qkv_proj columns and out_proj
rows split per head group. Each core computes a partial [D, T] output^T for
its batch; host sums the two group partials per batch, transposes, adds bias.

v2 vs baseline (396.8us): all matmuls in bf16 (1 cycle/row vs the ~2 c/r the
fp32r path measured on HW), x pre-transposed on the host (kills 128 PE
transposes + copies), everything SBUF-resident (no DRAM bounce of Q/K), the
1/sqrt(dh) scale folded into exp's scale immediate, and one unified emission
schedule: projection / output-projection units are interleaved as PE filler
between attention items so the PE never idles while the Scalar engine (exp,
~163us total, dtype-independent 1 elem/cycle/lane) grinds through softmax.

Per-core pipeline:
  proj units: Q^T/K^T[128(2 heads*64d), t] = W_pair.T @ x^T per (pair, 512-t
  chunk); V[t,d] natural per 128-t tile -> vS[k128, ktile, head, 65] with a
  ones column (softmax denominator via the AV matmul).
  attention items (2 k-tiles each): S^T[k,q] = K^T.T @ Q^T row-tiled 2 heads;
  exp(0.125*S) on ACT -> bf16 P^T; triangle mask-mul on diagonal blocks;
  AV: psum[65,512] += V'[k,d+1].T @ P^T accumulated over k-tiles (row 64 =
  denominator). Normalize with reciprocal + gpsimd partition_broadcast.
  o units: out^T[f,t] += Wo_pair[d128,f].T @ OT_pair[d128,t] over pairs.
PSUM: S pool 2x2 banks, AV 2x1, proj/outproj 2x1 = 8 banks.
"""

import numpy as np
import ml_dtypes

import concourse.bacc as bacc
import concourse.tile as tile
import concourse.mybir as mybir
from concourse import bass_utils
from concourse.bass import ts

F32 = mybir.dt.float32
BF16 = mybir.dt.bfloat16
EXP = mybir.ActivationFunctionType.Exp

T = 2048
TT = 16          # t tiles of 128
NP = 4           # head pairs per core
NQC = 4          # q chunks of 512
SCALE = 0.125    # 1/sqrt(64), folded into exp's scale immediate

_CACHE = {}
_last_in_maps = None


def _build(CT):
    """CT = number of 128-row c-tiles in the (possibly bias-augmented) x/W."""
    nc = bacc.Bacc("TRN2", target_bir_lowering=False, debug=False)
    C = CT * 128

    def mm(*args, **kwargs):
        return nc.tensor.matmul(*args, **kwargs)

    xa = nc.dram_tensor("xa", [C, T], BF16, kind="ExternalInput").ap()  # x^T
    wq = nc.dram_tensor("wq", [C, 512], BF16, kind="ExternalInput").ap()
    wk = nc.dram_tensor("wk", [C, 512], BF16, kind="ExternalInput").ap()
    wv = nc.dram_tensor("wv", [C, 512], BF16, kind="ExternalInput").ap()
    wo = nc.dram_tensor("wo", [512, 1024], BF16, kind="ExternalInput").ap()
    tri = nc.dram_tensor("tri", [128, 128], BF16, kind="ExternalInput").ap()
    ot = nc.dram_tensor("ot", [1024, T], F32, kind="ExternalOutput").ap()

    with tile.TileContext(nc) as tc:
        with (
            tc.tile_pool(name="persist", bufs=1) as persist,
            tc.tile_pool(name="ptp", bufs=6) as ptpool,
            tc.tile_pool(name="rsm", bufs=6) as rpool,
            tc.tile_pool(name="rbcp", bufs=2) as rbcpool,
            tc.tile_pool(name="obnc", bufs=2) as opool,
            tc.tile_pool(name="psS", bufs=2, space="PSUM") as psS,
            tc.tile_pool(name="psAv", bufs=2, space="PSUM") as psAv,
            tc.tile_pool(name="psP", bufs=2, space="PSUM") as psP,
        ):
            vS = persist.tile([128, TT, 8, 65], BF16)     # [k128, ktile, head, d+1]
            OT = persist.tile([128, NP, T], BF16)         # [d128(2 heads), pair, t]
            tr = persist.tile([128, 128], BF16)
            wo_sb = persist.tile([128, NP, 1024], BF16)
            qsb = persist.tile([128, NP, T], BF16)        # Q^T per pair
            ksb = persist.tile([128, NP, T], BF16)        # K^T per pair
            xt = persist.tile([128, CT, T], BF16)         # x^T tiles
            wq_sb = persist.tile([128, CT, NP, 128], BF16)
            wk_sb = persist.tile([128, CT, NP, 128], BF16)
            wv_sb = persist.tile([128, CT, 512], BF16)
            nc.vector.memset(vS[:, :, :, 64:65], 1.0)

            # ---- input DMAs: sync queue = early-needed, gpsimd = rest ----
            nc.sync.dma_start(
                out=xt[:, :, ts(0, 512)],
                in_=xa[:, ts(0, 512)].rearrange("(ct P) t -> P ct t", P=128),
            )
            nc.sync.dma_start(
                out=wq_sb,
                in_=wq.rearrange("(ct P) (np f) -> P ct np f", P=128, np=NP),
            )
            nc.sync.dma_start(
                out=wk_sb,
                in_=wk.rearrange("(ct P) (np f) -> P ct np f", P=128, np=NP),
            )
            nc.gpsimd.dma_start(out=tr, in_=tri)
            nc.gpsimd.dma_start(
                out=wv_sb, in_=wv.rearrange("(ct P) f -> P ct f", P=128)
            )
            for tc_ in range(1, 4):
                nc.sync.dma_start(
                    out=xt[:, :, ts(tc_, 512)],
                    in_=xa[:, ts(tc_, 512)].rearrange("(ct P) t -> P ct t", P=128),
                )
            nc.gpsimd.dma_start(
                out=wo_sb, in_=wo.rearrange("(np P) f -> P np f", P=128)
            )

            # ---------------- PE work units ----------------
            def qk_unit(p, tc_, which):
                w_sb = wq_sb if which == 0 else wk_sb
                dst = qsb if which == 0 else ksb
                ps = psP.tile([128, 512], F32, name="pp", tag="pp")
                for cc in range(CT):
                    mm(
                        ps,
                        lhsT=w_sb[:, cc, p, :],
                        rhs=xt[:, cc, ts(tc_, 512)],
                        start=(cc == 0),
                        stop=(cc == CT - 1),
                    )
                nc.vector.tensor_copy(out=dst[:, p, ts(tc_, 512)], in_=ps)

            def v_unit(tt):
                ps = psP.tile([128, 512], F32, name="pp", tag="pp")
                for cc in range(CT):
                    mm(
                        ps,
                        lhsT=xt[:, cc, ts(tt, 128)],
                        rhs=wv_sb[:, cc, :],
                        start=(cc == 0),
                        stop=(cc == CT - 1),
                    )
                nc.vector.tensor_copy(
                    out=vS[:, tt, :, 0:64],
                    in_=ps.rearrange("p (h d) -> p h d", h=8),
                )

            def o_unit(qc, ft):
                ps = psP.tile([128, 512], F32, name="pp", tag="pp")
                for p in range(NP):
                    mm(
                        ps,
                        lhsT=wo_sb[:, p, ts(ft, 128)],
                        rhs=OT[:, p, ts(qc, 512)],
                        start=(p == 0),
                        stop=(p == NP - 1),
                    )
                ob = opool.tile([128, 512], F32)
                nc.vector.tensor_copy(out=ob, in_=ps)
                nc.gpsimd.dma_start(out=ot[ts(ft, 128), ts(qc, 512)], in_=ob)

            # ---------------- attention ----------------
            pts = {}
            avs = {}

            def s_exp(p, qc, j):
                off = max(0, 128 * j - 512 * qc)
                sg = psS.tile([128, 2, 512], F32, name="sg", tag="sg")
                jo = 512 * (j // 4) + 128 * (j % 4)
                for m in range(2):
                    mm(
                        sg[:, m, off:],
                        lhsT=ksb[64 * m : 64 * m + 64, p, jo : jo + 128],
                        rhs=qsb[64 * m : 64 * m + 64, p, 512 * qc + off : 512 * qc + 512],
                        start=True,
                        stop=True,
                    )
                ptile = ptpool.tile([128, 2, 512], BF16, name="pt", tag="pt")
                nc.scalar.activation(
                    out=ptile[:, :, off:], in_=sg[:, :, off:], func=EXP, scale=SCALE
                )
                if j >= 4 * qc:
                    nc.vector.tensor_mul(
                        ptile[:, :, off : off + 128],
                        ptile[:, :, off : off + 128],
                        tr[:, None, :].to_broadcast([128, 2, 128]),
                    )
                pts[(p, qc, j)] = (ptile, off)

            def av_mm(p, qc, j, nj):
                ptile, off = pts.pop((p, qc, j))
                av = avs[(p, qc)]
                for m in range(2):
                    mm(
                        av[m][:65, off:],
                        lhsT=vS[:, j, 2 * p + m, :],
                        rhs=ptile[:, m, off:],
                        start=(j == 0),
                        stop=(j == nj - 1),
                    )

            def normalize(p, qc):
                av = avs.pop((p, qc))
                rrs = []
                for m in range(2):
                    rsb = rpool.tile([1, 512], F32, name="rsb", tag="rsb")
                    nc.vector.tensor_copy(out=rsb, in_=av[m][64:65, :])
                    # unnormalized O~ out of PSUM so the av bank frees fast
                    nc.vector.tensor_copy(
                        out=OT[64 * m : 64 * m + 64, p, ts(qc, 512)],
                        in_=av[m][0:64, :],
                    )
                    rrs.append(rsb)
                for m in range(2):
                    rinv = rpool.tile([1, 512], F32, name="rinv", tag="rinv")
                    nc.vector.reciprocal_approx_fast(out=rinv, in_=rrs[m])
                    rb16 = rpool.tile([1, 512], BF16, name="rb16", tag="rb16")
                    nc.vector.tensor_copy(out=rb16, in_=rinv)
                    rb = rbcpool.tile([128, 512], BF16, name="rb", tag="rb")
                    nc.gpsimd.partition_broadcast(rb, rb16)
                    sl = OT[64 * m : 64 * m + 64, p, ts(qc, 512)]
                    nc.vector.tensor_mul(sl, sl, rb[64 * m : 64 * m + 64, :])

            def av_item(it):
                p, qc, nj, jg, first, last = it
                if first:
                    avs[(p, qc)] = [
                        psAv.tile([128, 512], F32, name="av", tag="av")
                        for _ in range(2)
                    ]
                for j in jg:
                    av_mm(p, qc, j, nj)
                if last:
                    normalize(p, qc)

            # ---------------- emission schedule ----------------
            # items: 2 k-tiles of one (pair, q-chunk); rounds by q-chunk
            rounds = []
            for qc in range(NQC):
                ritems = []
                for p in range(NP):
                    nj = 4 * qc + 4
                    js = list(range(nj))
                    sub = [js[i : i + 2] for i in range(0, nj, 2)]
                    for gi, jg in enumerate(sub):
                        ritems.append((p, qc, nj, jg, gi == 0, gi == len(sub) - 1))
                rounds.append(ritems)

            # per-round PE filler units (produce data for round r+1; drain
            # outproj of round r-1). Order within a round matters: producers
            # must precede their consumers in PE emission order, and o_units
            # of round r-1 must come after normalize(p3, r-1), which is only
            # emitted during item 1 of round r (AV lag) -> late list.
            fillers_early = [
                # round 0: finish r0 prereqs first (v2,v3 then qk for p1-3),
                # then round-1 projections
                [(v_unit, (tt,)) for tt in (2, 3)]
                + [(qk_unit, (p, 0, w)) for p in (1, 2, 3) for w in (0, 1)]
                + [(qk_unit, (p, 1, w)) for p in range(NP) for w in (0, 1)]
                + [(v_unit, (tt,)) for tt in (4, 5, 6, 7)],
                [(qk_unit, (p, 2, w)) for p in range(NP) for w in (0, 1)]
                + [(v_unit, (tt,)) for tt in (8, 9, 10, 11)],
                [(qk_unit, (p, 3, w)) for p in range(NP) for w in (0, 1)]
                + [(v_unit, (tt,)) for tt in (12, 13, 14, 15)],
                [],
            ]
            fillers_late = [
                [],
                [(o_unit, (0, ft)) for ft in range(8)],
                [(o_unit, (1, ft)) for ft in range(8)],
                [(o_unit, (2, ft)) for ft in range(8)],
            ]
            LATE_START = 4  # item index within the round where late fillers may begin

            # prologue: just enough for the first items of pair 0
            qk_unit(0, 0, 0)
            qk_unit(0, 0, 1)
            v_unit(0)
            v_unit(1)

            LAG = 2
            all_items = []

            def do_item(it):
                p, qc, nj, jg, first, last = it
                for j in jg:
                    s_exp(p, qc, j)
                all_items.append(it)
                k = len(all_items) - 1
                if k - LAG >= 0:
                    av_item(all_items[k - LAG])

            for r in range(NQC):
                ritems = rounds[r]
                fe, fle = fillers_early[r], fillers_late[r]
                n_it, ne, nl = len(ritems), len(fe), len(fle)
                ei = li = 0
                for ii, it in enumerate(ritems):
                    do_item(it)
                    ewant = ((ii + 1) * ne + n_it - 1) // n_it
                    while ei < min(ewant, ne):
                        fn, args = fe[ei]
                        fn(*args)
                        ei += 1
                    if ii >= LATE_START:
                        lwant = ((ii + 1 - LATE_START) * nl + (n_it - LATE_START) - 1) // max(
                            1, n_it - LATE_START
                        )
                        while li < min(lwant, nl):
                            fn, args = fle[li]
                            fn(*args)
                            li += 1
                while ei < ne:
                    fn, args = fe[ei]
                    fn(*args)
                    ei += 1
                while li < nl:
                    fn, args = fle[li]
                    fn(*args)
                    li += 1

            # drain the AV lag, then the last output-projection round
            for k in range(len(all_items) - LAG, len(all_items)):
                av_item(all_items[k])
            for ft in range(8):
                o_unit(3, ft)

    nc.compile()
    return nc


def kernel(x, W_qkv, b_qkv, W_out, b_out):
    global _last_in_maps
    bf = ml_dtypes.bfloat16
    x = np.asarray(x, dtype=np.float32)
    W_qkv = np.asarray(W_qkv, dtype=np.float32)
    b_qkv = np.asarray(b_qkv, dtype=np.float32)
    W_out = np.asarray(W_out, dtype=np.float32)
    b_out = np.asarray(b_out, dtype=np.float32)
    B = x.shape[0]

    aug = bool(np.any(b_qkv))
    CT = 9 if aug else 8
    if CT not in _CACHE:
        _CACHE[CT] = _build(CT)
    nc = _CACHE[CT]

    # triangle keep-mask for the diagonal 128 block: [p, c] = 1 if c >= p
    tri = (np.arange(128)[None, :] >= np.arange(128)[:, None]).astype(bf)

    in_maps = []
    for core in range(8):
        b, g = core // 2, core % 2
        xa = x[b]
        if aug:
            pad = np.zeros((T, 128), np.float32)
            pad[:, 0] = 1.0
            xa = np.concatenate([xa, pad], axis=1)

        def wslice(col0):
            w = W_qkv[:, col0 + 512 * g : col0 + 512 * g + 512]
            if aug:
                extra = np.zeros((128, 512), np.float32)
                extra[0] = b_qkv[col0 + 512 * g : col0 + 512 * g + 512]
                w = np.concatenate([w, extra], axis=0)
            return np.ascontiguousarray(w.astype(bf))

        in_maps.append(
            {
                "xa": np.ascontiguousarray(xa.T.astype(bf)),
                "wq": wslice(0),
                "wk": wslice(1024),
                "wv": wslice(2048),
                "wo": np.ascontiguousarray(
                    W_out[512 * g : 512 * g + 512, :].astype(bf)
                ),
                "tri": tri,
            }
        )

    _last_in_maps = in_maps
    res = bass_utils.run_bass_kernel_spmd(nc, in_maps, list(range(8))).results
    out = np.empty((B, T, 1024), np.float32)
    for b in range(B):
        acc = res[2 * b]["ot"] + res[2 * b + 1]["ot"]
        out[b] = acc.T + b_out[None, :]
    return out


# revision 13
# speedup vs baseline: 1.4474x; 1.0182x over previous
"""Causal self-attention (B=4, T=2048, D=1024, H=16) on 8 TRN2 NeuronCores.

Sharding: core i = (batch b = i//2, head-group g = i%2). Data parallel on B,
tensor parallel on heads (8 heads per group): qkv_proj columns and out_proj
rows split per head group. Each core computes a partial [D, T] output^T for
its batch; host sums the two group partials per batch, transposes, adds bias.

v2 vs baseline (396.8us): all matmuls in bf16 (1 cycle/row vs the ~2 c/r the
fp32r path measured on HW), x pre-transposed on the host (kills 128 PE
transposes + copies), everything SBUF-resident (no DRAM bounce of Q/K), the
1/sqrt(dh) scale folded into exp's scale immediate, and one unified emission
schedule: projection / output-projection units are interleaved as PE filler
between attention items so the PE never idles while the Scalar engine (exp,
~163us total, dtype-independent 1 elem/cycle/lane) grinds through softmax.

Per-core pipeline:
  proj units: Q^T/K^T[128(2 heads*64d), t] = W_pair.T @ x^T per (pair, 512-t
  chunk); V[t,d] natural per 128-t tile -> vS[k128, ktile, head, 65] with a
  ones column (softmax denominator via the AV matmul).
  attention items (2 k-tiles each): S^T[k,q] = K^T.T @ Q^T row-tiled 2 heads;
  exp(0.125*S) on ACT -> bf16 P^T; triangle mask-mul on diagonal blocks;
  AV: psum[65,512] += V'[k,d+1].T @ P^T accumulated over k-tiles (row 64 =
  denominator). Normalize with reciprocal + gpsimd partition_broadcast.
  o units: out^T[f,t] += Wo_pair[d128,f].T @ OT_pair[d128,t] over pairs.
PSUM: S pool 2x2 banks, AV 2x1, proj/outproj 2x1 = 8 banks.
"""

import numpy as np
import ml_dtypes

import concourse.bacc as bacc
import concourse.tile as tile
import concourse.mybir as mybir
from concourse import bass_utils
from concourse.bass import ts

F32 = mybir.dt.float32
BF16 = mybir.dt.bfloat16
EXP = mybir.ActivationFunctionType.Exp

T = 2048
TT = 16          # t tiles of 128
NP = 4           # head pairs per core
NQC = 4          # q chunks of 512
SCALE = 0.125    # 1/sqrt(64), folded into exp's scale immediate

_CACHE = {}
_last_in_maps = None


def _build(CT):
    """CT = number of 128-row c-tiles in the (possibly bias-augmented) x/W."""
    nc = bacc.Bacc("TRN2", target_bir_lowering=False, debug=False)
    C = CT * 128

    def mm(*args, **kwargs):
        return nc.tensor.matmul(*args, **kwargs)

    xa = nc.dram_tensor("xa", [C, T], BF16, kind="ExternalInput").ap()  # x^T
    wq = nc.dram_tensor("wq", [C, 512], BF16, kind="ExternalInput").ap()
    wk = nc.dram_tensor("wk", [C, 512], BF16, kind="ExternalInput").ap()
    wv = nc.dram_tensor("wv", [C, 512], BF16, kind="ExternalInput").ap()
    wo = nc.dram_tensor("wo", [512, 1024], BF16, kind="ExternalInput").ap()
    tri = nc.dram_tensor("tri", [128, 128], BF16, kind="ExternalInput").ap()
    ot = nc.dram_tensor("ot", [1024, T], F32, kind="ExternalOutput").ap()

    with tile.TileContext(nc) as tc:
        with (
            tc.tile_pool(name="persist", bufs=1) as persist,
            tc.tile_pool(name="ptp", bufs=6) as ptpool,
            tc.tile_pool(name="rsm", bufs=6) as rpool,
            tc.tile_pool(name="rbcp", bufs=2) as rbcpool,
            tc.tile_pool(name="obnc", bufs=3) as opool,
            tc.tile_pool(name="psS", bufs=2, space="PSUM") as psS,
            tc.tile_pool(name="psAv", bufs=2, space="PSUM") as psAv,
            tc.tile_pool(name="psP", bufs=2, space="PSUM") as psP,
        ):
            vS = persist.tile([128, TT, 8, 65], BF16)     # [k128, ktile, head, d+1]
            OT = persist.tile([128, NP, T], BF16)         # [d128(2 heads), pair, t]
            tr = persist.tile([128, 128], BF16)
            wo_sb = persist.tile([128, NP, 1024], BF16)
            qsb = persist.tile([128, NP, T], BF16)        # Q^T per pair
            ksb = persist.tile([128, NP, T], BF16)        # K^T per pair
            xt = persist.tile([128, CT, T], BF16)         # x^T tiles
            wq_sb = persist.tile([128, CT, 512], BF16)
            wk_sb = persist.tile([128, CT, 512], BF16)
            wv_sb = persist.tile([128, CT, 512], BF16)
            nc.vector.memset(vS[:, :, :, 64:65], 1.0)

            # ---- input DMAs, spread across queues so the first qk_unit's
            # deps (xt chunk 0, wq, wk) land ~3us in, not serialized ----
            nc.sync.dma_start(
                out=xt[:, :, ts(0, 512)],
                in_=xa[:, ts(0, 512)].rearrange("(ct P) t -> P ct t", P=128),
            )
            nc.scalar.dma_start(
                out=wq_sb, in_=wq.rearrange("(ct P) f -> P ct f", P=128)
            )
            nc.gpsimd.dma_start(
                out=wk_sb, in_=wk.rearrange("(ct P) f -> P ct f", P=128)
            )
            nc.gpsimd.dma_start(out=tr, in_=tri)
            nc.gpsimd.dma_start(
                out=wv_sb, in_=wv.rearrange("(ct P) f -> P ct f", P=128)
            )
            for tc_ in range(1, 4):
                nc.sync.dma_start(
                    out=xt[:, :, ts(tc_, 512)],
                    in_=xa[:, ts(tc_, 512)].rearrange("(ct P) t -> P ct t", P=128),
                )
            nc.gpsimd.dma_start(
                out=wo_sb, in_=wo.rearrange("(np P) f -> P np f", P=128)
            )

            # ---------------- PE work units ----------------
            def qk_unit(p, tc_, which):
                w_sb = wq_sb if which == 0 else wk_sb
                dst = qsb if which == 0 else ksb
                ps = psP.tile([128, 512], F32, name="pp", tag="pp")
                for cc in range(CT):
                    mm(
                        ps,
                        lhsT=w_sb[:, cc, ts(p, 128)],
                        rhs=xt[:, cc, ts(tc_, 512)],
                        start=(cc == 0),
                        stop=(cc == CT - 1),
                    )
                nc.vector.tensor_copy(out=dst[:, p, ts(tc_, 512)], in_=ps)

            def v_unit(tt):
                ps = psP.tile([128, 512], F32, name="pp", tag="pp")
                for cc in range(CT):
                    mm(
                        ps,
                        lhsT=xt[:, cc, ts(tt, 128)],
                        rhs=wv_sb[:, cc, :],
                        start=(cc == 0),
                        stop=(cc == CT - 1),
                    )
                nc.vector.tensor_copy(
                    out=vS[:, tt, :, 0:64],
                    in_=ps.rearrange("p (h d) -> p h d", h=8),
                )

            def o_unit(qc, ft):
                ps = psP.tile([128, 512], F32, name="pp", tag="pp")
                for p in range(NP):
                    mm(
                        ps,
                        lhsT=wo_sb[:, p, ts(ft, 128)],
                        rhs=OT[:, p, ts(qc, 512)],
                        start=(p == 0),
                        stop=(p == NP - 1),
                    )
                ob = opool.tile([128, 512], F32)
                nc.vector.tensor_copy(out=ob, in_=ps)
                # alternate queues; sync is idle after the input loads
                eng = nc.sync if ft % 2 == 0 else nc.gpsimd
                eng.dma_start(out=ot[ts(ft, 128), ts(qc, 512)], in_=ob)

            # ---------------- attention ----------------
            pts = {}
            avs = {}

            def s_exp(p, qc, j):
                off = max(0, 128 * j - 512 * qc)
                sg = psS.tile([128, 2, 512], F32, name="sg", tag="sg")
                jo = 512 * (j // 4) + 128 * (j % 4)
                for m in range(2):
                    mm(
                        sg[:, m, off:],
                        lhsT=ksb[64 * m : 64 * m + 64, p, jo : jo + 128],
                        rhs=qsb[64 * m : 64 * m + 64, p, 512 * qc + off : 512 * qc + 512],
                        start=True,
                        stop=True,
                    )
                ptile = ptpool.tile([128, 2, 512], BF16, name="pt", tag="pt")
                nc.scalar.activation(
                    out=ptile[:, :, off:], in_=sg[:, :, off:], func=EXP, scale=SCALE
                )
                if j >= 4 * qc:
                    nc.vector.tensor_mul(
                        ptile[:, :, off : off + 128],
                        ptile[:, :, off : off + 128],
                        tr[:, None, :].to_broadcast([128, 2, 128]),
                    )
                pts[(p, qc, j)] = (ptile, off)

            def av_mm(p, qc, j, nj):
                ptile, off = pts.pop((p, qc, j))
                av = avs[(p, qc)]
                for m in range(2):
                    mm(
                        av[m][:65, off:],
                        lhsT=vS[:, j, 2 * p + m, :],
                        rhs=ptile[:, m, off:],
                        start=(j == 0),
                        stop=(j == nj - 1),
                    )

            def normalize(p, qc):
                av = avs.pop((p, qc))
                rrs = []
                for m in range(2):
                    rsb = rpool.tile([1, 512], F32, name="rsb", tag="rsb")
                    nc.vector.tensor_copy(out=rsb, in_=av[m][64:65, :])
                    # unnormalized O~ out of PSUM so the av bank frees fast
                    nc.vector.tensor_copy(
                        out=OT[64 * m : 64 * m + 64, p, ts(qc, 512)],
                        in_=av[m][0:64, :],
                    )
                    rrs.append(rsb)
                for m in range(2):
                    rinv = rpool.tile([1, 512], F32, name="rinv", tag="rinv")
                    nc.vector.reciprocal_approx_fast(out=rinv, in_=rrs[m])
                    rb16 = rpool.tile([1, 512], BF16, name="rb16", tag="rb16")
                    nc.vector.tensor_copy(out=rb16, in_=rinv)
                    rb = rbcpool.tile([128, 512], BF16, name="rb", tag="rb")
                    nc.gpsimd.partition_broadcast(rb, rb16)
                    sl = OT[64 * m : 64 * m + 64, p, ts(qc, 512)]
                    nc.vector.tensor_mul(sl, sl, rb[64 * m : 64 * m + 64, :])

            def av_item(it):
                p, qc, nj, jg, first, last = it
                if first:
                    avs[(p, qc)] = [
                        psAv.tile([128, 512], F32, name="av", tag="av")
                        for _ in range(2)
                    ]
                for j in jg:
                    av_mm(p, qc, j, nj)
                if last:
                    normalize(p, qc)

            # ---------------- emission schedule ----------------
            # items: 2 k-tiles of one (pair, q-chunk); rounds by q-chunk
            rounds = []
            for qc in range(NQC):
                ritems = []
                for p in range(NP):
                    nj = 4 * qc + 4
                    js = list(range(nj))
                    sub = [js[i : i + 2] for i in range(0, nj, 2)]
                    for gi, jg in enumerate(sub):
                        ritems.append((p, qc, nj, jg, gi == 0, gi == len(sub) - 1))
                rounds.append(ritems)

            # per-round PE filler units (produce data for round r+1; drain
            # outproj of round r-1). Order within a round matters: producers
            # must precede their consumers in PE emission order, and o_units
            # of round r-1 must come after normalize(p3, r-1), which is only
            # emitted during item 1 of round r (AV lag) -> late list.
            fillers_early = [
                # round 0: r0 prereqs at the exact slots they are consumed
                # (pair p's qk before item 2p; vS tile j before its AV),
                # then round-1 projections
                [(v_unit, (0,)), (v_unit, (1,)),
                 (qk_unit, (1, 0, 0)), (qk_unit, (1, 0, 1)), (v_unit, (2,)),
                 (qk_unit, (2, 0, 0)), (qk_unit, (2, 0, 1)), (v_unit, (3,)),
                 (qk_unit, (3, 0, 0)), (qk_unit, (3, 0, 1))]
                + [(qk_unit, (p, 1, w)) for p in range(NP) for w in (0, 1)]
                + [(v_unit, (tt,)) for tt in (4, 5, 6, 7)],
                [(qk_unit, (p, 2, w)) for p in range(NP) for w in (0, 1)]
                + [(v_unit, (tt,)) for tt in (8, 9, 10, 11)],
                [(qk_unit, (p, 3, w)) for p in range(NP) for w in (0, 1)]
                + [(v_unit, (tt,)) for tt in (12, 13, 14, 15)],
                [],
            ]
            fillers_late = [
                [],
                [(o_unit, (0, ft)) for ft in range(8)],
                [(o_unit, (1, ft)) for ft in range(8)],
                [(o_unit, (2, ft)) for ft in range(8)],
            ]
            LATE_START = 4  # item index within the round where late fillers may begin

            # prologue: just enough for the first items of pair 0 (vS tiles
            # 0/1 are first fillers -- AV lags 2 items, so they land in time)
            qk_unit(0, 0, 0)
            qk_unit(0, 0, 1)

            LAG = 2
            all_items = []

            def do_item(it):
                p, qc, nj, jg, first, last = it
                for j in jg:
                    s_exp(p, qc, j)
                all_items.append(it)
                k = len(all_items) - 1
                if k - LAG >= 0:
                    av_item(all_items[k - LAG])

            for r in range(NQC):
                ritems = rounds[r]
                fe, fle = fillers_early[r], fillers_late[r]
                n_it, ne, nl = len(ritems), len(fe), len(fle)
                ei = li = 0
                for ii, it in enumerate(ritems):
                    do_item(it)
                    ewant = ((ii + 1) * ne + n_it - 1) // n_it
                    while ei < min(ewant, ne):
                        fn, args = fe[ei]
                        fn(*args)
                        ei += 1
                    if ii >= LATE_START:
                        lwant = ((ii + 1 - LATE_START) * nl + (n_it - LATE_START) - 1) // max(
                            1, n_it - LATE_START
                        )
                        while li < min(lwant, nl):
                            fn, args = fle[li]
                            fn(*args)
                            li += 1
                while ei < ne:
                    fn, args = fe[ei]
                    fn(*args)
                    ei += 1
                while li < nl:
                    fn, args = fle[li]
                    fn(*args)
                    li += 1

            # drain the AV lag, then the last output-projection round
            for k in range(len(all_items) - LAG, len(all_items)):
                av_item(all_items[k])
            for ft in range(8):
                o_unit(3, ft)

    nc.compile()
    return nc


def kernel(x, W_qkv, b_qkv, W_out, b_out):
    global _last_in_maps
    bf = ml_dtypes.bfloat16
    x = np.asarray(x, dtype=np.float32)
    W_qkv = np.asarray(W_qkv, dtype=np.float32)
    b_qkv = np.asarray(b_qkv, dtype=np.float32)
    W_out = np.asarray(W_out, dtype=np.float32)
    b_out = np.asarray(b_out, dtype=np.float32)
    B = x.shape[0]

    aug = bool(np.any(b_qkv))
    CT = 9 if aug else 8
    if CT not in _CACHE:
        _CACHE[CT] = _build(CT)
    nc = _CACHE[CT]

    # triangle keep-mask for the diagonal 128 block: [p, c] = 1 if c >= p
    tri = (np.arange(128)[None, :] >= np.arange(128)[:, None]).astype(bf)

    in_maps = []
    for core in range(8):
        b, g = core // 2, core % 2
        xa = x[b]
        if aug:
            pad = np.zeros((T, 128), np.float32)
            pad[:, 0] = 1.0
            xa = np.concatenate([xa, pad], axis=1)

        def wslice(col0):
            w = W_qkv[:, col0 + 512 * g : col0 + 512 * g + 512]
            if aug:
                extra = np.zeros((128, 512), np.float32)
                extra[0] = b_qkv[col0 + 512 * g : col0 + 512 * g + 512]
                w = np.concatenate([w, extra], axis=0)
            return np.ascontiguousarray(w.astype(bf))

        in_maps.append(
            {
                "xa": np.ascontiguousarray(xa.T.astype(bf)),
                "wq": wslice(0),
                "wk": wslice(1024),
                "wv": wslice(2048),
                "wo": np.ascontiguousarray(
                    W_out[512 * g : 512 * g + 512, :].astype(bf)
                ),
                "tri": tri,
            }
        )

    _last_in_maps = in_maps
    res = bass_utils.run_bass_kernel_spmd(nc, in_maps, list(range(8))).results
    out = np.empty((B, T, 1024), np.float32)
    for b in range(B):
        acc = res[2 * b]["ot"] + res[2 * b + 1]["ot"]
        out[b] = acc.T + b_out[None, :]
    return out


# revision 17
# speedup vs baseline: 1.4509x; 1.0024x over previous
"""Causal self-attention (B=4, T=2048, D=1024, H=16) on 8 TRN2 NeuronCores.

Sharding: core i = (batch b = i//2, head-group g = i%2). Data parallel on B,
tensor parallel on heads (8 heads per group): qkv_proj columns and out_proj
rows split per head group. Each core computes a partial [D, T] output^T for
its batch; host sums the two group partials per batch, transposes, adds bias.

v2 vs baseline (396.8us): all matmuls in bf16 (1 cycle/row vs the ~2 c/r the
fp32r path measured on HW), x pre-transposed on the host (kills 128 PE
transposes + copies), everything SBUF-resident (no DRAM bounce of Q/K), the
1/sqrt(dh) scale folded into exp's scale immediate, and one unified emission
schedule: projection / output-projection units are interleaved as PE filler
between attention items so the PE never idles while the Scalar engine (exp,
~163us total, dtype-independent 1 elem/cycle/lane) grinds through softmax.

Per-core pipeline:
  proj units: Q^T/K^T[128(2 heads*64d), t] = W_pair.T @ x^T per (pair, 512-t
  chunk); V[t,d] natural per 128-t tile -> vS[k128, ktile, head, 65] with a
  ones column (softmax denominator via the AV matmul).
  attention items (2 k-tiles each): S^T[k,q] = K^T.T @ Q^T row-tiled 2 heads;
  exp(0.125*S) on ACT -> bf16 P^T; triangle mask-mul on diagonal blocks;
  AV: psum[65,512] += V'[k,d+1].T @ P^T accumulated over k-tiles (row 64 =
  denominator). Normalize with reciprocal + gpsimd partition_broadcast.
  o units: out^T[f,t] += Wo_pair[d128,f].T @ OT_pair[d128,t] over pairs.
PSUM: S pool 2x2 banks, AV 2x1, proj/outproj 2x1 = 8 banks.
"""

import numpy as np
import ml_dtypes

import concourse.bacc as bacc
import concourse.tile as tile
import concourse.mybir as mybir
from concourse import bass_utils
from concourse.bass import ts

F32 = mybir.dt.float32
BF16 = mybir.dt.bfloat16
EXP = mybir.ActivationFunctionType.Exp

T = 2048
TT = 16          # t tiles of 128
NP = 4           # head pairs per core
NQC = 4          # q chunks of 512
SCALE = 0.125    # 1/sqrt(64), folded into exp's scale immediate

_CACHE = {}
_last_in_maps = None


def _build(CT):
    """CT = number of 128-row c-tiles in the (possibly bias-augmented) x/W."""
    nc = bacc.Bacc("TRN2", target_bir_lowering=False, debug=False)
    C = CT * 128

    def mm(*args, **kwargs):
        return nc.tensor.matmul(*args, **kwargs)

    xa = nc.dram_tensor("xa", [C, T], BF16, kind="ExternalInput").ap()  # x^T
    wq = nc.dram_tensor("wq", [C, 512], BF16, kind="ExternalInput").ap()
    wk = nc.dram_tensor("wk", [C, 512], BF16, kind="ExternalInput").ap()
    wv = nc.dram_tensor("wv", [C, 512], BF16, kind="ExternalInput").ap()
    wo = nc.dram_tensor("wo", [512, 1024], BF16, kind="ExternalInput").ap()
    tri = nc.dram_tensor("tri", [128, 128], BF16, kind="ExternalInput").ap()
    ot = nc.dram_tensor("ot", [1024, T], F32, kind="ExternalOutput").ap()

    with tile.TileContext(nc) as tc:
        with (
            tc.tile_pool(name="persist", bufs=1) as persist,
            tc.tile_pool(name="ptp", bufs=6) as ptpool,
            tc.tile_pool(name="rsm", bufs=6) as rpool,
            tc.tile_pool(name="rbcp", bufs=2) as rbcpool,
            tc.tile_pool(name="obnc", bufs=3) as opool,
            tc.tile_pool(name="psS", bufs=2, space="PSUM") as psS,
            tc.tile_pool(name="psAv", bufs=2, space="PSUM") as psAv,
            tc.tile_pool(name="psP", bufs=2, space="PSUM") as psP,
        ):
            vS = persist.tile([128, TT, 8, 65], BF16)     # [k128, ktile, head, d+1]
            OT = persist.tile([128, NP, T], BF16)         # [d128(2 heads), pair, t]
            tr = persist.tile([128, 128], BF16)
            wo_sb = persist.tile([128, NP, 1024], BF16)
            qsb = persist.tile([128, NP, T], BF16)        # Q^T per pair
            ksb = persist.tile([128, NP, T], BF16)        # K^T per pair
            xt = persist.tile([128, CT, T], BF16)         # x^T tiles
            wq_sb = persist.tile([128, CT, 512], BF16)
            wk_sb = persist.tile([128, CT, 512], BF16)
            wv_sb = persist.tile([128, CT, 512], BF16)
            nc.vector.memset(vS[:, :, :, 64:65], 1.0)

            # ---- input DMAs. HBM bw (~358GB/s) is shared by every transfer
            # in flight, so the critical first-unit deps (xt chunk 0, the
            # pair-0 columns of wq/wk) are issued alone; all bulk transfers
            # are gated behind a dummy DVE read of their target regions that
            # only runs after the prologue's psum copies (see below).
            nc.sync.dma_start(
                out=xt[:, :, ts(0, 512)],
                in_=xa[:, ts(0, 512)].rearrange("(ct P) t -> P ct t", P=128),
            )
            nc.scalar.dma_start(
                out=wq_sb[:, :, 0:128],
                in_=wq[:, 0:128].rearrange("(ct P) f -> P ct f", P=128),
            )
            nc.gpsimd.dma_start(
                out=wk_sb[:, :, 0:128],
                in_=wk[:, 0:128].rearrange("(ct P) f -> P ct f", P=128),
            )
            nc.gpsimd.dma_start(out=tr, in_=tri)

            # ---------------- PE work units ----------------
            def qk_unit(p, tc_, which):
                w_sb = wq_sb if which == 0 else wk_sb
                dst = qsb if which == 0 else ksb
                ps = psP.tile([128, 512], F32, name="pp", tag="pp")
                for cc in range(CT):
                    mm(
                        ps,
                        lhsT=w_sb[:, cc, ts(p, 128)],
                        rhs=xt[:, cc, ts(tc_, 512)],
                        start=(cc == 0),
                        stop=(cc == CT - 1),
                    )
                nc.vector.tensor_copy(out=dst[:, p, ts(tc_, 512)], in_=ps)

            def v_unit(tt):
                ps = psP.tile([128, 512], F32, name="pp", tag="pp")
                for cc in range(CT):
                    mm(
                        ps,
                        lhsT=xt[:, cc, ts(tt, 128)],
                        rhs=wv_sb[:, cc, :],
                        start=(cc == 0),
                        stop=(cc == CT - 1),
                    )
                nc.vector.tensor_copy(
                    out=vS[:, tt, :, 0:64],
                    in_=ps.rearrange("p (h d) -> p h d", h=8),
                )

            def o_unit(qc, ft):
                ps = psP.tile([128, 512], F32, name="pp", tag="pp")
                for p in range(NP):
                    mm(
                        ps,
                        lhsT=wo_sb[:, p, ts(ft, 128)],
                        rhs=OT[:, p, ts(qc, 512)],
                        start=(p == 0),
                        stop=(p == NP - 1),
                    )
                ob = opool.tile([128, 512], F32)
                nc.vector.tensor_copy(out=ob, in_=ps)
                # alternate queues; sync is idle after the input loads
                eng = nc.sync if ft % 2 == 0 else nc.gpsimd
                eng.dma_start(out=ot[ts(ft, 128), ts(qc, 512)], in_=ob)

            # ---------------- attention ----------------
            pts = {}
            avs = {}

            def s_exp(p, qc, j):
                off = max(0, 128 * j - 512 * qc)
                sg = psS.tile([128, 2, 512], F32, name="sg", tag="sg")
                jo = 512 * (j // 4) + 128 * (j % 4)
                for m in range(2):
                    mm(
                        sg[:, m, off:],
                        lhsT=ksb[64 * m : 64 * m + 64, p, jo : jo + 128],
                        rhs=qsb[64 * m : 64 * m + 64, p, 512 * qc + off : 512 * qc + 512],
                        start=True,
                        stop=True,
                    )
                ptile = ptpool.tile([128, 2, 512], BF16, name="pt", tag="pt")
                nc.scalar.activation(
                    out=ptile[:, :, off:], in_=sg[:, :, off:], func=EXP, scale=SCALE
                )
                if j >= 4 * qc:
                    nc.vector.tensor_mul(
                        ptile[:, :, off : off + 128],
                        ptile[:, :, off : off + 128],
                        tr[:, None, :].to_broadcast([128, 2, 128]),
                    )
                pts[(p, qc, j)] = (ptile, off)

            def av_mm(p, qc, j, nj):
                ptile, off = pts.pop((p, qc, j))
                av = avs[(p, qc)]
                for m in range(2):
                    mm(
                        av[m][:65, off:],
                        lhsT=vS[:, j, 2 * p + m, :],
                        rhs=ptile[:, m, off:],
                        start=(j == 0),
                        stop=(j == nj - 1),
                    )

            def normalize(p, qc):
                av = avs.pop((p, qc))
                rrs = []
                for m in range(2):
                    rsb = rpool.tile([1, 512], F32, name="rsb", tag="rsb")
                    nc.vector.tensor_copy(out=rsb, in_=av[m][64:65, :])
                    # unnormalized O~ out of PSUM so the av bank frees fast
                    nc.vector.tensor_copy(
                        out=OT[64 * m : 64 * m + 64, p, ts(qc, 512)],
                        in_=av[m][0:64, :],
                    )
                    rrs.append(rsb)
                for m in range(2):
                    rinv = rpool.tile([1, 512], F32, name="rinv", tag="rinv")
                    nc.vector.reciprocal_approx_fast(out=rinv, in_=rrs[m])
                    rb16 = rpool.tile([1, 512], BF16, name="rb16", tag="rb16")
                    nc.vector.tensor_copy(out=rb16, in_=rinv)
                    rb = rbcpool.tile([128, 512], BF16, name="rb", tag="rb")
                    nc.gpsimd.partition_broadcast(rb, rb16)
                    sl = OT[64 * m : 64 * m + 64, p, ts(qc, 512)]
                    nc.vector.tensor_mul(sl, sl, rb[64 * m : 64 * m + 64, :])

            def av_item(it):
                p, qc, nj, jg, first, last = it
                if first:
                    avs[(p, qc)] = [
                        psAv.tile([128, 512], F32, name="av", tag="av")
                        for _ in range(2)
                    ]
                for j in jg:
                    av_mm(p, qc, j, nj)
                if last:
                    normalize(p, qc)

            # ---------------- emission schedule ----------------
            # items: 2 k-tiles of one (pair, q-chunk); rounds by q-chunk
            rounds = []
            for qc in range(NQC):
                ritems = []
                for p in range(NP):
                    nj = 4 * qc + 4
                    js = list(range(nj))
                    sub = [js[i : i + 2] for i in range(0, nj, 2)]
                    for gi, jg in enumerate(sub):
                        ritems.append((p, qc, nj, jg, gi == 0, gi == len(sub) - 1))
                rounds.append(ritems)

            # per-round PE filler units (produce data for round r+1; drain
            # outproj of round r-1). Order within a round matters: producers
            # must precede their consumers in PE emission order, and o_units
            # of round r-1 must come after normalize(p3, r-1), which is only
            # emitted during item 1 of round r (AV lag) -> late list.
            fillers_early = [
                # round 0: r0 prereqs at the exact slots they are consumed
                # (pair p's qk before item 2p; vS tile j before its AV),
                # then round-1 projections
                [(v_unit, (0,)), (v_unit, (1,)),
                 (qk_unit, (1, 0, 0)), (qk_unit, (1, 0, 1)), (v_unit, (2,)),
                 (qk_unit, (2, 0, 0)), (qk_unit, (2, 0, 1)), (v_unit, (3,)),
                 (qk_unit, (3, 0, 0)), (qk_unit, (3, 0, 1))]
                + [(qk_unit, (p, 1, w)) for p in range(NP) for w in (0, 1)]
                + [(v_unit, (tt,)) for tt in (4, 5, 6, 7)],
                [(qk_unit, (p, 2, w)) for p in range(NP) for w in (0, 1)]
                + [(v_unit, (tt,)) for tt in (8, 9, 10, 11)],
                [(qk_unit, (p, 3, w)) for p in range(NP) for w in (0, 1)]
                + [(v_unit, (tt,)) for tt in (12, 13, 14, 15)],
                [],
            ]
            fillers_late = [
                [],
                [(o_unit, (0, ft)) for ft in range(8)],
                [(o_unit, (1, ft)) for ft in range(8)],
                [(o_unit, (2, ft)) for ft in range(8)],
            ]
            LATE_START = 4  # item index within the round where late fillers may begin

            # prologue: just enough for the first items of pair 0 (vS tiles
            # 0/1 are first fillers -- AV lags 2 items, so they land in time)
            qk_unit(0, 0, 0)
            qk_unit(0, 0, 1)

            # bulk DMAs, gated so they start only once the critical loads are
            # done: a dummy read of each target region sits on the DVE queue
            # behind the prologue's psum copies, and each DMA write must wait
            # for that read (WAR) before touching HBM bandwidth.
            gate = rpool.tile([1, 8], BF16, name="gate", tag="gate")
            nc.vector.tensor_copy(out=gate[:, 0:1], in_=xt[0:1, 0, 512:513])
            nc.vector.tensor_copy(out=gate[:, 1:2], in_=wq_sb[0:1, 0, 128:129])
            nc.vector.tensor_copy(out=gate[:, 2:3], in_=wk_sb[0:1, 0, 128:129])
            nc.vector.tensor_copy(out=gate[:, 3:4], in_=wv_sb[0:1, 0, 0:1])
            nc.vector.tensor_copy(out=gate[:, 4:5], in_=wo_sb[0:1, 0, 0:1])
            nc.scalar.dma_start(
                out=wq_sb[:, :, 128:512],
                in_=wq[:, 128:512].rearrange("(ct P) f -> P ct f", P=128),
            )
            nc.gpsimd.dma_start(
                out=wk_sb[:, :, 128:512],
                in_=wk[:, 128:512].rearrange("(ct P) f -> P ct f", P=128),
            )
            nc.sync.dma_start(
                out=wv_sb, in_=wv.rearrange("(ct P) f -> P ct f", P=128)
            )
            for tc_ in range(1, 4):
                nc.sync.dma_start(
                    out=xt[:, :, ts(tc_, 512)],
                    in_=xa[:, ts(tc_, 512)].rearrange("(ct P) t -> P ct t", P=128),
                )
            nc.gpsimd.dma_start(
                out=wo_sb, in_=wo.rearrange("(np P) f -> P np f", P=128)
            )

            LAG = 2
            all_items = []

            def do_item(it):
                p, qc, nj, jg, first, last = it
                for j in jg:
                    s_exp(p, qc, j)
                all_items.append(it)
                k = len(all_items) - 1
                if k - LAG >= 0:
                    av_item(all_items[k - LAG])

            for r in range(NQC):
                ritems = rounds[r]
                fe, fle = fillers_early[r], fillers_late[r]
                n_it, ne, nl = len(ritems), len(fe), len(fle)
                ei = li = 0
                for ii, it in enumerate(ritems):
                    do_item(it)
                    ewant = ((ii + 1) * ne + n_it - 1) // n_it
                    while ei < min(ewant, ne):
                        fn, args = fe[ei]
                        fn(*args)
                        ei += 1
                    if ii >= LATE_START:
                        lwant = ((ii + 1 - LATE_START) * nl + (n_it - LATE_START) - 1) // max(
                            1, n_it - LATE_START
                        )
                        while li < min(lwant, nl):
                            fn, args = fle[li]
                            fn(*args)
                            li += 1
                while ei < ne:
                    fn, args = fe[ei]
                    fn(*args)
                    ei += 1
                while li < nl:
                    fn, args = fle[li]
                    fn(*args)
                    li += 1

            # drain the AV lag, then the last output-projection round
            for k in range(len(all_items) - LAG, len(all_items)):
                av_item(all_items[k])
            for ft in range(8):
                o_unit(3, ft)

    nc.compile()
    return nc


def kernel(x, W_qkv, b_qkv, W_out, b_out):
    global _last_in_maps
    bf = ml_dtypes.bfloat16
    x = np.asarray(x, dtype=np.float32)
    W_qkv = np.asarray(W_qkv, dtype=np.float32)
    b_qkv = np.asarray(b_qkv, dtype=np.float32)
    W_out = np.asarray(W_out, dtype=np.float32)
    b_out = np.asarray(b_out, dtype=np.float32)
    B = x.shape[0]

    aug = bool(np.any(b_qkv))
    CT = 9 if aug else 8
    if CT not in _CACHE:
        _CACHE[CT] = _build(CT)
    nc = _CACHE[CT]

    # triangle keep-mask for the diagonal 128 block: [p, c] = 1 if c >= p
    tri = (np.arange(128)[None, :] >= np.arange(128)[:, None]).astype(bf)

    in_maps = []
    for core in range(8):
        b, g = core // 2, core % 2
        xa = x[b]
        if aug:
            pad = np.zeros((T, 128), np.float32)
            pad[:, 0] = 1.0
            xa = np.concatenate([xa, pad], axis=1)

        def wslice(col0):
            w = W_qkv[:, col0 + 512 * g : col0 + 512 * g + 512]
            if aug:
                extra = np.zeros((128, 512), np.float32)
                extra[0] = b_qkv[col0 + 512 * g : col0 + 512 * g + 512]
                w = np.concatenate([w, extra], axis=0)
            return np.ascontiguousarray(w.astype(bf))

        in_maps.append(
            {
                "xa": np.ascontiguousarray(xa.T.astype(bf)),
                "wq": wslice(0),
                "wk": wslice(1024),
                "wv": wslice(2048),
                "wo": np.ascontiguousarray(
                    W_out[512 * g : 512 * g + 512, :].astype(bf)
                ),
                "tri": tri,
            }
        )

    _last_in_maps = in_maps
    res = bass_utils.run_bass_kernel_spmd(nc, in_maps, list(range(8))).results
    out = np.empty((B, T, 1024), np.float32)
    for b in range(B):
        acc = res[2 * b]["ot"] + res[2 * b + 1]["ot"]
        out[b] = acc.T + b_out[None, :]
    return out
